# revision 10
# baseline (speedup 1.0000x reference)
"""Trainium2 Bass kernel for the dual-branch CustomLSTMCell.

Math (reference):
    hx_l = [h_light | y]  [B, H+I]     hx_t = [h_temp | y]
    z_br = hx_br @ W_br.T + b_br       (W_br = vstack(w_f,w_i,w_c,w_o) [4H, H+I])
    f,i,ch,o = sigmoid/sigmoid/tanh/sigmoid splits of z_br
    c_new = (f1 + f2) * c_light + i1*ch1 + i2*ch2      (c_temp is unused)
    h_new = (o1 + o2) * tanh(c_new)

Strategy: 2D shard over 8 NeuronCores — 4-way batch x 2-way hidden, no
collectives (each core owns a disjoint (batch, hidden) tile of h/c, and
gate row j only needs the hx rows the core already has). Per core:
batch 1024 (2 moving chunks of 512), hidden 512 per gate (4 row-tiles).

The GEMMs run in fp16 (1 PE cycle/row; fp32 would be 4x slower, and
fp8-DoubleRow's 2x FLOP rate cannot pay for the 3x FLOPs a split-precision
scheme needs to stay under the accuracy gate — measured 271us vs this
design's PE floor of 164us). Per core we compute z.T tiles: psum[zcol 128,
batch 512] = Wtile[K=128, M=128].T @ hxT[K=128, N=512], accumulated over
K=1536 (12 k-tiles). vs pure data-parallel, the 2D shard halves the weight
stream (12.6 MB/core fp16), so DMA (~21 MB/core total) sits well under the
PE time and stays fully overlapped.

Gate bias + sigmoid/tanh run on the Scalar engine straight out of PSUM
(bias is per-partition in this transposed layout), the LSTM cell
elementwise runs on the Vector engine in fp32, results DMA out transposed,
and the host un-transposes. All transposes/casts happen host-side so every
device DMA is a contiguous 2D partition-major stream.

Schedule notes: weight DMAs are issued with a software prefetch distance
of PREFETCH units; the first weight unit is split into k-chunks so the
first matmul only waits on a 33KB slice. A short PE pre-warm (dummy
matmuls on a zeroed tile) burns the DVFS p-state ramp while the first
operands are in flight. The per-branch gate order is (i, c, f, o) so the
output gate of the temp branch — the only input of the final h_new chain —
finishes last and the tail after the final matmul is short.
"""

import os
import sys

for _p in ("/opt/trn_rl_repo",):
    if os.path.isdir(_p) and _p not in sys.path:
        sys.path.append(_p)

import numpy as np

import concourse.bass as bass
import concourse.mybir as mybir
import concourse.tile as tile
from concourse import bacc
from concourse.bass_utils import run_bass_kernel_spmd

B, I, H = 4096, 512, 1024
N_CORES = 8
SB, SH = 4, 2              # batch x hidden core grid
BS = B // SB               # 1024 batch rows per core
CH = 2                     # batch chunks per core
CB = BS // CH              # 512 moving rows per matmul
HS = H // SH               # 512 hidden outputs per gate per core
RT = HS // 128             # 4 hidden row-tiles
K = H + I                  # 1536 contraction
KT = K // 128              # 12 k-tiles
N_U = RT * 2 * 4           # 32 weight units: (r, branch, gate)
GATE_ORDER = (1, 2, 0, 3)  # i, c, f, o
PREFETCH = 5               # weight units in flight ahead of use
N_WARM = 24                # PE pre-warm matmuls (512-cycle each)

_F32 = mybir.dt.float32
_F16 = mybir.dt.float16
AF = mybir.ActivationFunctionType
F16 = np.float16


def _build_nc():
    nc = bacc.Bacc("TRN2", target_bir_lowering=False, debug=False,
                   enable_asserts=False)

    wp = nc.dram_tensor("wp", [N_U, 128, KT * 128], _F16,
                        kind="ExternalInput")
    ap = nc.dram_tensor("ap", [2, CH, 128, KT * CB], _F16,
                        kind="ExternalInput")
    bp = nc.dram_tensor("bp", [128, N_U], _F32, kind="ExternalInput")
    ct = nc.dram_tensor("ct", [RT, CH, 128, CB], _F32, kind="ExternalInput")
    h_out = nc.dram_tensor("h_out", [RT, CH, 128, CB], _F32,
                           kind="ExternalOutput")
    c_out = nc.dram_tensor("c_out", [RT, CH, 128, CB], _F32,
                           kind="ExternalOutput")

    with tile.TileContext(nc) as tc:
        with (
            tc.tile_pool(name="const", bufs=1) as cpool,
            tc.tile_pool(name="w", bufs=17) as wpool,
            tc.tile_pool(name="gates", bufs=18) as gpool,
            tc.tile_pool(name="cin", bufs=4) as cin_pool,
            tc.tile_pool(name="ew", bufs=4) as epool,
            tc.tile_pool(name="out", bufs=4) as opool,
            tc.tile_pool(name="psum", bufs=8, space="PSUM") as pspool,
        ):
            wt_tiles = {}

            def issue_wt(seq, eng=None):
                t = wpool.tile([128, KT * 128], _F16, tag="w")
                (eng or nc.sync).dma_start(out=t[:], in_=wp[seq])
                wt_tiles[seq] = t

            # PE pre-warm: dummy matmuls on a zeroed tile burn the DVFS
            # p-state ramp while the first operands are in flight.
            warm = cpool.tile([128, 512], _F16, tag="warm")
            nc.gpsimd.memset(warm[:], 0.0)
            wpsum = pspool.tile([128, 512], _F32, tag="pt")
            for _ in range(N_WARM):
                nc.tensor.matmul(wpsum[:], warm[:, 0:128], warm[:],
                                 start=True, stop=True)

            # startup. Queue discipline: the sync queue carries ONLY the
            # weight stream (so weight unit u never waits behind bulk
            # activation bytes); gpsimd and scalar queues split the
            # activation tiles, bias, c_light, and the output writes.
            wt0 = wpool.tile([128, KT * 128], _F16, tag="w")
            for k in range(KT):
                nc.sync.dma_start(out=wt0[:, bass.ts(k, 128)],
                                  in_=wp[0][:, bass.ts(k, 128)])
            wt_tiles[0] = wt0
            issue_wt(1)
            a_sb = {}
            for br in range(2):
                for ch in range(CH):
                    a_tile = cpool.tile([128, KT * CB], _F16,
                                        tag=f"a{br}{ch}")
                    a_sb[(br, ch)] = a_tile
            bias_sb = cpool.tile([128, N_U], _F32, tag="bias")
            nc.scalar.dma_start(out=bias_sb[:], in_=bp[:])
            # a(0,0) is on the critical path: round-robin its k-chunks
            # over all three queues so it lands within ~3us of queue start.
            a00 = a_sb[(0, 0)]
            qs = (nc.gpsimd, nc.scalar, nc.sync)
            for k in range(KT):
                qs[k % 3].dma_start(out=a00[:, bass.ts(k, CB)],
                                    in_=ap[0, 0][:, bass.ts(k, CB)])
            for u in range(2, 8):
                issue_wt(u)
            nc.gpsimd.dma_start(out=a_sb[(1, 0)][:], in_=ap[1, 0])
            nc.scalar.dma_start(out=a_sb[(0, 1)][:], in_=ap[0, 1])
            nc.scalar.dma_start(out=a_sb[(1, 1)][:], in_=ap[1, 1])

            for r in range(RT):
                ct_t = []
                for ch in range(CH):
                    t = cin_pool.tile([128, CB], _F32, tag="ct")
                    eng = nc.gpsimd if ch == 0 else nc.scalar
                    eng.dma_start(out=t[:], in_=ct[r, ch])
                    ct_t.append(t)

                gates = {}
                # chunk-phased: all 8 units of this row-tile run chunk 0
                # first, then the (still resident) units run chunk 1. This
                # halves the activation bytes the DMA queues must deliver
                # before the PE's first stall-free stretch.
                for ch in range(CH):
                    for br in range(2):
                        for gi, g in enumerate(GATE_ORDER):
                            seq = (r * 2 + br) * 4 + gi
                            if ch == 0 and seq + 8 < N_U:
                                issue_wt(seq + 8)
                            wt = (wt_tiles.pop(seq) if ch == CH - 1
                                  else wt_tiles[seq])
                            func = AF.Tanh if g == 2 else AF.Sigmoid
                            a_t = a_sb[(br, ch)]
                            pt = pspool.tile([128, CB], _F32, tag="pt")
                            for k in range(KT):
                                nc.tensor.matmul(
                                    pt[:],
                                    wt[:, bass.ts(k, 128)],
                                    a_t[:, bass.ts(k, CB)],
                                    start=(k == 0),
                                    stop=(k == KT - 1),
                                )
                            gt = gpool.tile([128, CB], _F32, tag="gate")
                            nc.scalar.activation(gt[:], pt[:], func,
                                                 bias=bias_sb[:, seq:seq + 1],
                                                 scale=1.0)
                            gates[(br, g, ch)] = gt

                    f1, i1, ch1, o1 = (gates[(0, g, ch)] for g in range(4))
                    f2, i2, ch2, o2 = (gates[(1, g, ch)] for g in range(4))

                    t_a = epool.tile([128, CB], _F32, tag="ta")
                    t_b = epool.tile([128, CB], _F32, tag="tb")
                    t_c = epool.tile([128, CB], _F32, tag="tc")
                    c_new = opool.tile([128, CB], _F32, tag="cn")
                    nc.vector.tensor_mul(t_b[:], i1[:], ch1[:])
                    nc.vector.tensor_mul(t_c[:], i2[:], ch2[:])
                    nc.vector.tensor_add(t_b[:], t_b[:], t_c[:])
                    nc.vector.tensor_add(t_a[:], f1[:], f2[:])
                    nc.vector.tensor_mul(t_a[:], t_a[:], ct_t[ch][:])
                    nc.vector.tensor_add(c_new[:], t_a[:], t_b[:])

                    th = epool.tile([128, CB], _F32, tag="th")
                    nc.scalar.activation(th[:], c_new[:], AF.Tanh)
                    h_new = opool.tile([128, CB], _F32, tag="hn")
                    nc.vector.tensor_add(t_a[:], o1[:], o2[:])
                    nc.vector.tensor_mul(h_new[:], t_a[:], th[:])

                    last = (r == RT - 1)
                    if last:
                        # parallelize the tail writes across idle queues
                        hb = CB // 2
                        nc.gpsimd.dma_start(out=c_out[r, ch][:, 0:hb],
                                            in_=c_new[:, 0:hb])
                        nc.sync.dma_start(out=c_out[r, ch][:, hb:CB],
                                          in_=c_new[:, hb:CB])
                        nc.scalar.dma_start(out=h_out[r, ch][:, 0:hb],
                                            in_=h_new[:, 0:hb])
                        nc.sync.dma_start(out=h_out[r, ch][:, hb:CB],
                                          in_=h_new[:, hb:CB])
                    else:
                        nc.gpsimd.dma_start(out=c_out[r, ch], in_=c_new[:])
                        nc.scalar.dma_start(out=h_out[r, ch], in_=h_new[:])

    nc.compile()
    return nc


_NC_CACHE = None


def _get_nc():
    global _NC_CACHE
    if _NC_CACHE is None:
        _NC_CACHE = _build_nc()
    return _NC_CACHE


def _pack_weights(inputs, hid):
    """-> wp [N_U, 128, KT*128] f16, bp [128, N_U] f32 for hidden shard
    `hid` (shared by the 4 batch-shard cores in that column).

    Unit seq order matches the device loop: (r, br, g in GATE_ORDER).
    Unit layout per partition row kk: [k(KT), m(128)] = 3072B, where
    wp[u][kk, k*128 + m] = W[u_row m, k*128 + kk].
    """
    rs = slice(hid * HS, (hid + 1) * HS)
    units = np.empty((RT, 2, 4, 128, KT, 128), dtype=F16)
    bps = np.empty((128, RT, 2, 4), dtype=np.float32)
    for bi, suffix in enumerate(("_light", "_light_temp")):
        for gi, g in enumerate(GATE_ORDER):
            gname = "fico"[g]
            Wg = inputs[f"w_{gname}{suffix}"][rs].astype(F16)   # [HS, K]
            bg = inputs[f"b_{gname}{suffix}"][rs]
            t = Wg.reshape(RT, 128, KT, 128)                    # [r, m, k, kk]
            units[:, bi, gi] = t.transpose(0, 3, 2, 1)          # [r, kk, k, m]
            bps[:, :, bi, gi] = bg.reshape(RT, 128).T
    wp = np.ascontiguousarray(units).reshape(N_U, 128, KT * 128)
    bp = np.ascontiguousarray(bps).reshape(128, N_U)
    return wp, bp


def _pack_core_inputs(inputs, wps, bps, core):
    b4, hid = divmod(core, SH)
    bsl = slice(b4 * BS, (b4 + 1) * BS)
    y = inputs["y"][bsl]
    out = {"wp": wps[hid], "bp": bps[hid]}
    a = np.empty((2, CH, 128, KT * CB), dtype=F16)
    for bi, hname in ((0, "h_light"), (1, "h_temp")):
        hx = np.concatenate([inputs[hname][bsl], y], axis=1).astype(F16)
        # [ch, p, k, j]: element = hx[ch*CB + j, k*128 + p]
        a2 = hx.reshape(CH, CB, KT, 128).transpose(0, 3, 2, 1)
        a[bi] = np.ascontiguousarray(a2).reshape(CH, 128, KT * CB)
    out["ap"] = a
    cl = inputs["c_light"][bsl, hid * HS:(hid + 1) * HS].astype(np.float32)
    # ct[r, ch, p, j] = c[ch*CB + j, r*128 + p]
    c4 = cl.reshape(CH, CB, RT, 128).transpose(2, 0, 3, 1)
    out["ct"] = np.ascontiguousarray(c4)
    return out


def make_in_maps(**inputs):
    wps, bps = zip(*[_pack_weights(inputs, hid) for hid in range(SH)])
    return [_pack_core_inputs(inputs, wps, bps, c) for c in range(N_CORES)]


def _unpack_core(res):
    # [r, ch, p, j] -> [ch*CB + j, r*128 + p]
    h = res["h_out"].transpose(1, 3, 0, 2).reshape(BS, HS)
    c = res["c_out"].transpose(1, 3, 0, 2).reshape(BS, HS)
    return h, c


def unpack_core0(res0):
    return _unpack_core(res0)


def unpack_results(results):
    h_new = np.empty((B, H), dtype=np.float32)
    c_new = np.empty((B, H), dtype=np.float32)
    for core, res in enumerate(results):
        b4, hid = divmod(core, SH)
        h, c = _unpack_core(res)
        h_new[b4 * BS:(b4 + 1) * BS, hid * HS:(hid + 1) * HS] = h
        c_new[b4 * BS:(b4 + 1) * BS, hid * HS:(hid + 1) * HS] = c
    return h_new, c_new


def kernel(**inputs):
    inputs = {k: np.asarray(v) for k, v in inputs.items()}
    nc = _get_nc()
    in_maps = make_in_maps(**inputs)
    res = run_bass_kernel_spmd(nc, in_maps, list(range(N_CORES)))
    return unpack_results(res.results)


# revision 11
# speedup vs baseline: 1.0199x; 1.0199x over previous
"""Trainium2 Bass kernel for the dual-branch CustomLSTMCell.

Math (reference):
    hx_l = [h_light | y]  [B, H+I]     hx_t = [h_temp | y]
    z_br = hx_br @ W_br.T + b_br       (W_br = vstack(w_f,w_i,w_c,w_o) [4H, H+I])
    f,i,ch,o = sigmoid/sigmoid/tanh/sigmoid splits of z_br
    c_new = (f1 + f2) * c_light + i1*ch1 + i2*ch2      (c_temp is unused)
    h_new = (o1 + o2) * tanh(c_new)

Strategy: 2D shard over 8 NeuronCores — 4-way batch x 2-way hidden, no
collectives (each core owns a disjoint (batch, hidden) tile of h/c, and
gate row j only needs the hx rows the core already has). Per core:
batch 1024 (2 moving chunks of 512), hidden 512 per gate (4 row-tiles).

The GEMMs run in fp16 (1 PE cycle/row; fp32 would be 4x slower, and
fp8-DoubleRow's 2x FLOP rate cannot pay for the 3x FLOPs a split-precision
scheme needs to stay under the accuracy gate — measured 271us vs this
design's PE floor of 164us). Per core we compute z.T tiles: psum[zcol 128,
batch 512] = Wtile[K=128, M=128].T @ hxT[K=128, N=512], accumulated over
K=1536 (12 k-tiles). vs pure data-parallel, the 2D shard halves the weight
stream (12.6 MB/core fp16), so DMA (~21 MB/core total) sits well under the
PE time and stays fully overlapped.

Gate bias + sigmoid/tanh run on the Scalar engine straight out of PSUM
(bias is per-partition in this transposed layout), the LSTM cell
elementwise runs on the Vector engine in fp32, results DMA out transposed,
and the host un-transposes. All transposes/casts happen host-side so every
device DMA is a contiguous 2D partition-major stream.

Schedule notes: weight DMAs are issued with a software prefetch distance
of PREFETCH units; the first weight unit is split into k-chunks so the
first matmul only waits on a 33KB slice. A short PE pre-warm (dummy
matmuls on a zeroed tile) burns the DVFS p-state ramp while the first
operands are in flight. The per-branch gate order is (i, c, f, o) so the
output gate of the temp branch — the only input of the final h_new chain —
finishes last and the tail after the final matmul is short.
"""

import os
import sys

for _p in ("/opt/trn_rl_repo",):
    if os.path.isdir(_p) and _p not in sys.path:
        sys.path.append(_p)

import numpy as np

import concourse.bass as bass
import concourse.mybir as mybir
import concourse.tile as tile
from concourse import bacc
from concourse.bass_utils import run_bass_kernel_spmd

B, I, H = 4096, 512, 1024
N_CORES = 8
SB, SH = 4, 2              # batch x hidden core grid
BS = B // SB               # 1024 batch rows per core
CH = 2                     # batch chunks per core
CB = BS // CH              # 512 moving rows per matmul
HS = H // SH               # 512 hidden outputs per gate per core
RT = HS // 128             # 4 hidden row-tiles
K = H + I                  # 1536 contraction
KT = K // 128              # 12 k-tiles
N_U = RT * 2 * 4           # 32 weight units: (r, branch, gate)
GATE_ORDER = (1, 2, 0, 3)  # i, c, f, o
PREFETCH = 5               # weight units in flight ahead of use
N_WARM = 12                # PE pre-warm matmuls (512-cycle each)

_F32 = mybir.dt.float32
_F16 = mybir.dt.float16
AF = mybir.ActivationFunctionType
F16 = np.float16


def _build_nc():
    nc = bacc.Bacc("TRN2", target_bir_lowering=False, debug=False,
                   enable_asserts=False)

    WCOL = KT * 128 + 8        # weight unit cols: 12 k-tiles + bias col
    wp = nc.dram_tensor("wp", [N_U, 128, WCOL], _F16,
                        kind="ExternalInput")
    ap = nc.dram_tensor("ap", [2, CH, 128, KT * CB], _F16,
                        kind="ExternalInput")
    ct = nc.dram_tensor("ct", [RT, CH, 128, CB], _F32, kind="ExternalInput")
    h_out = nc.dram_tensor("h_out", [RT, CH, 128, CB], _F32,
                           kind="ExternalOutput")
    c_out = nc.dram_tensor("c_out", [RT, CH, 128, CB], _F32,
                           kind="ExternalOutput")

    with tile.TileContext(nc) as tc:
        with (
            tc.tile_pool(name="const", bufs=1) as cpool,
            tc.tile_pool(name="w", bufs=17) as wpool,
            tc.tile_pool(name="gates", bufs=18) as gpool,
            tc.tile_pool(name="cin", bufs=4) as cin_pool,
            tc.tile_pool(name="ew", bufs=4) as epool,
            tc.tile_pool(name="out", bufs=4) as opool,
            tc.tile_pool(name="psum", bufs=8, space="PSUM") as pspool,
        ):
            wt_tiles = {}

            def issue_wt(seq, eng=None):
                t = wpool.tile([128, WCOL], _F16, tag="w")
                (eng or nc.sync).dma_start(out=t[:], in_=wp[seq])
                wt_tiles[seq] = t

            # PE pre-warm: dummy matmuls on a zeroed tile burn the DVFS
            # p-state ramp while the first operands are in flight.
            warm = cpool.tile([128, 512], _F16, tag="warm")
            nc.gpsimd.memset(warm[:], 0.0)
            wpsum = pspool.tile([128, 512], _F32, tag="pt")
            for _ in range(N_WARM):
                nc.tensor.matmul(wpsum[:], warm[:, 0:128], warm[:],
                                 start=True, stop=True)

            # startup. DMA-queue economics: each DMA costs one descriptor
            # per partition row (~15ns dispatch each), so only wide-row
            # transfers move data fast. The sync queue carries ONLY the
            # weight stream (whole 3KB-row units, first unit in two halves);
            # gpsimd + scalar queues split the activation tiles (6KB-row
            # halves for the critical first tile), c_light, and outputs.
            wt0 = wpool.tile([128, WCOL], _F16, tag="w")
            hk = KT // 2
            nc.sync.dma_start(out=wt0[:, 0:hk * 128],
                              in_=wp[0][:, 0:hk * 128])
            nc.sync.dma_start(out=wt0[:, hk * 128:WCOL],
                              in_=wp[0][:, hk * 128:WCOL])
            wt_tiles[0] = wt0
            a_sb = {}
            for br in range(2):
                for ch in range(CH):
                    a_tile = cpool.tile([128, KT * CB], _F16,
                                        tag=f"a{br}{ch}")
                    a_sb[(br, ch)] = a_tile
            a00 = a_sb[(0, 0)]
            nc.gpsimd.dma_start(out=a00[:, 0:hk * CB],
                                in_=ap[0, 0][:, 0:hk * CB])
            nc.scalar.dma_start(out=a00[:, hk * CB:KT * CB],
                                in_=ap[0, 0][:, hk * CB:KT * CB])
            for u in range(1, 8):
                issue_wt(u)
            nc.gpsimd.dma_start(out=a_sb[(1, 0)][:], in_=ap[1, 0])
            nc.scalar.dma_start(out=a_sb[(0, 1)][:], in_=ap[0, 1])
            nc.scalar.dma_start(out=a_sb[(1, 1)][:], in_=ap[1, 1])

            for r in range(RT):
                ct_t = []
                for ch in range(CH):
                    t = cin_pool.tile([128, CB], _F32, tag="ct")
                    eng = nc.gpsimd if ch == 0 else nc.scalar
                    eng.dma_start(out=t[:], in_=ct[r, ch])
                    ct_t.append(t)

                gates = {}
                # chunk-phased: all 8 units of this row-tile run chunk 0
                # first, then the (still resident) units run chunk 1. This
                # halves the activation bytes the DMA queues must deliver
                # before the PE's first stall-free stretch.
                for ch in range(CH):
                    for br in range(2):
                        for gi, g in enumerate(GATE_ORDER):
                            seq = (r * 2 + br) * 4 + gi
                            if ch == 0 and seq + 8 < N_U:
                                issue_wt(seq + 8)
                            wt = (wt_tiles.pop(seq) if ch == CH - 1
                                  else wt_tiles[seq])
                            func = AF.Tanh if g == 2 else AF.Sigmoid
                            a_t = a_sb[(br, ch)]
                            pt = pspool.tile([128, CB], _F32, tag="pt")
                            for k in range(KT):
                                nc.tensor.matmul(
                                    pt[:],
                                    wt[:, bass.ts(k, 128)],
                                    a_t[:, bass.ts(k, CB)],
                                    start=(k == 0),
                                    stop=(k == KT - 1),
                                )
                            gt = gpool.tile([128, CB], _F32, tag="gate")
                            bcol = KT * 128
                            nc.scalar.activation(gt[:], pt[:], func,
                                                 bias=wt[:, bcol:bcol + 1],
                                                 scale=1.0)
                            gates[(br, g, ch)] = gt

                    f1, i1, ch1, o1 = (gates[(0, g, ch)] for g in range(4))
                    f2, i2, ch2, o2 = (gates[(1, g, ch)] for g in range(4))

                    t_a = epool.tile([128, CB], _F32, tag="ta")
                    t_b = epool.tile([128, CB], _F32, tag="tb")
                    t_c = epool.tile([128, CB], _F32, tag="tc")
                    c_new = opool.tile([128, CB], _F32, tag="cn")
                    nc.vector.tensor_mul(t_b[:], i1[:], ch1[:])
                    nc.vector.tensor_mul(t_c[:], i2[:], ch2[:])
                    nc.vector.tensor_add(t_b[:], t_b[:], t_c[:])
                    nc.vector.tensor_add(t_a[:], f1[:], f2[:])
                    nc.vector.tensor_mul(t_a[:], t_a[:], ct_t[ch][:])
                    nc.vector.tensor_add(c_new[:], t_a[:], t_b[:])

                    th = epool.tile([128, CB], _F32, tag="th")
                    nc.scalar.activation(th[:], c_new[:], AF.Tanh)
                    h_new = opool.tile([128, CB], _F32, tag="hn")
                    nc.vector.tensor_add(t_a[:], o1[:], o2[:])
                    nc.vector.tensor_mul(h_new[:], t_a[:], th[:])

                    last = (r == RT - 1)
                    if last:
                        # parallelize the tail writes across idle queues
                        hb = CB // 2
                        nc.gpsimd.dma_start(out=c_out[r, ch][:, 0:hb],
                                            in_=c_new[:, 0:hb])
                        nc.sync.dma_start(out=c_out[r, ch][:, hb:CB],
                                          in_=c_new[:, hb:CB])
                        nc.scalar.dma_start(out=h_out[r, ch][:, 0:hb],
                                            in_=h_new[:, 0:hb])
                        nc.sync.dma_start(out=h_out[r, ch][:, hb:CB],
                                          in_=h_new[:, hb:CB])
                    else:
                        nc.gpsimd.dma_start(out=c_out[r, ch], in_=c_new[:])
                        nc.scalar.dma_start(out=h_out[r, ch], in_=h_new[:])

    nc.compile()
    return nc


_NC_CACHE = None


def _get_nc():
    global _NC_CACHE
    if _NC_CACHE is None:
        _NC_CACHE = _build_nc()
    return _NC_CACHE


def _pack_weights(inputs, hid):
    """-> wp [N_U, 128, KT*128+8] f16 for hidden shard `hid` (shared by
    the 4 batch-shard cores in that column). Column KT*128 of each unit
    carries the gate bias for that unit's 128 output rows.

    Unit seq order matches the device loop: (r, br, g in GATE_ORDER).
    Unit layout per partition row kk: [k(KT), m(128)] then bias, where
    wp[u][kk, k*128 + m] = W[u_row m, k*128 + kk].
    """
    rs = slice(hid * HS, (hid + 1) * HS)
    units = np.zeros((RT, 2, 4, 128, KT * 128 + 8), dtype=F16)
    for bi, suffix in enumerate(("_light", "_light_temp")):
        for gi, g in enumerate(GATE_ORDER):
            gname = "fico"[g]
            Wg = inputs[f"w_{gname}{suffix}"][rs].astype(F16)   # [HS, K]
            bg = inputs[f"b_{gname}{suffix}"][rs].astype(F16)
            t = Wg.reshape(RT, 128, KT, 128)                    # [r, m, k, kk]
            units[:, bi, gi, :, :KT * 128] = t.transpose(0, 3, 2, 1).reshape(
                RT, 128, KT * 128)
            units[:, bi, gi, :, KT * 128] = bg.reshape(RT, 128)
    return np.ascontiguousarray(units).reshape(N_U, 128, KT * 128 + 8)


def _pack_core_inputs(inputs, wps, core):
    b4, hid = divmod(core, SH)
    bsl = slice(b4 * BS, (b4 + 1) * BS)
    y = inputs["y"][bsl]
    out = {"wp": wps[hid]}
    a = np.empty((2, CH, 128, KT * CB), dtype=F16)
    for bi, hname in ((0, "h_light"), (1, "h_temp")):
        hx = np.concatenate([inputs[hname][bsl], y], axis=1).astype(F16)
        # [ch, p, k, j]: element = hx[ch*CB + j, k*128 + p]
        a2 = hx.reshape(CH, CB, KT, 128).transpose(0, 3, 2, 1)
        a[bi] = np.ascontiguousarray(a2).reshape(CH, 128, KT * CB)
    out["ap"] = a
    cl = inputs["c_light"][bsl, hid * HS:(hid + 1) * HS].astype(np.float32)
    # ct[r, ch, p, j] = c[ch*CB + j, r*128 + p]
    c4 = cl.reshape(CH, CB, RT, 128).transpose(2, 0, 3, 1)
    out["ct"] = np.ascontiguousarray(c4)
    return out


def make_in_maps(**inputs):
    wps = [_pack_weights(inputs, hid) for hid in range(SH)]
    return [_pack_core_inputs(inputs, wps, c) for c in range(N_CORES)]


def _unpack_core(res):
    # [r, ch, p, j] -> [ch*CB + j, r*128 + p]
    h = res["h_out"].transpose(1, 3, 0, 2).reshape(BS, HS)
    c = res["c_out"].transpose(1, 3, 0, 2).reshape(BS, HS)
    return h, c


def unpack_core0(res0):
    return _unpack_core(res0)


def unpack_results(results):
    h_new = np.empty((B, H), dtype=np.float32)
    c_new = np.empty((B, H), dtype=np.float32)
    for core, res in enumerate(results):
        b4, hid = divmod(core, SH)
        h, c = _unpack_core(res)
        h_new[b4 * BS:(b4 + 1) * BS, hid * HS:(hid + 1) * HS] = h
        c_new[b4 * BS:(b4 + 1) * BS, hid * HS:(hid + 1) * HS] = c
    return h_new, c_new


def kernel(**inputs):
    inputs = {k: np.asarray(v) for k, v in inputs.items()}
    nc = _get_nc()
    in_maps = make_in_maps(**inputs)
    res = run_bass_kernel_spmd(nc, in_maps, list(range(N_CORES)))
    return unpack_results(res.results)


# revision 12
# speedup vs baseline: 1.0724x; 1.0514x over previous
"""Trainium2 Bass kernel for the dual-branch CustomLSTMCell.

Math (reference):
    hx_l = [h_light | y]  [B, H+I]     hx_t = [h_temp | y]
    z_br = hx_br @ W_br.T + b_br       (W_br = vstack(w_f,w_i,w_c,w_o) [4H, H+I])
    f,i,ch,o = sigmoid/sigmoid/tanh/sigmoid splits of z_br
    c_new = (f1 + f2) * c_light + i1*ch1 + i2*ch2      (c_temp is unused)
    h_new = (o1 + o2) * tanh(c_new)

Strategy: pure data-parallel over 8 NeuronCores — batch 4096 -> 8 x 512,
weights replicated. Per core we compute z.T tiles: psum[zcol 128, batch 512]
= Wtile[K=128, M=128].T @ hxT[K=128, N=512], accumulated over K=1536 (12
k-tiles) in fp16 (1 PE cycle/row; fp32 would be 4x slower; fp8-DoubleRow's
2x FLOP rate cannot pay for the 3x FLOPs a split-precision scheme needs to
stay under the accuracy gate — measured 271us vs this design's 164us PE
floor). Gate bias + sigmoid/tanh run on the Scalar engine straight out of
PSUM (bias rides in the last column of each weight unit, so it needs no
separate descriptor-heavy DMA), the LSTM cell elementwise runs on the
Vector engine in fp32, results DMA out transposed, host un-transposes.

DMA-queue economics drive the schedule: a DMA costs one descriptor per
partition row (~15ns dispatch each), so only wide-row transfers are fast,
and the queues deliver slowly for the first ~15us. The weight stream is
split across the sync and scalar queues (even/odd units, 3KB rows, first
units halved so matmul k=0 waits on 0.2MB), activations move as 6KB-row
half-tiles, c_light and the output writes ride the gpsimd queue, and the
final row-tile's outputs fan out over all three queues to shorten the
tail. A PE pre-warm (dummy matmuls) burns the DVFS p-state ramp while the
first operands are in flight. The per-branch gate order is (i, c, f, o) so
the output gate of the temp branch — the only input of the final h_new
chain — finishes last and the tail after the final matmul is short.
"""

import os
import sys

for _p in ("/opt/trn_rl_repo",):
    if os.path.isdir(_p) and _p not in sys.path:
        sys.path.append(_p)

import numpy as np

import concourse.bass as bass
import concourse.mybir as mybir
import concourse.tile as tile
from concourse import bacc
from concourse.bass_utils import run_bass_kernel_spmd

B, I, H = 4096, 512, 1024
N_CORES = 8
BS = B // N_CORES          # 512 batch rows per core
RT = H // 128              # 8 hidden row-tiles
K = H + I                  # 1536 contraction
KT = K // 128              # 12 k-tiles
N_U = RT * 2 * 4           # 64 weight units: (r, branch, gate)
WCOL = KT * 128 + 8        # weight unit cols: 12 k-tiles + bias col
GATE_ORDER = (1, 2, 0, 3)  # i, c, f, o
PREFETCH = 8               # weight units in flight ahead of use
N_WARM = 10                # PE pre-warm matmuls (512-cycle each)

_F32 = mybir.dt.float32
_F16 = mybir.dt.float16
AF = mybir.ActivationFunctionType
F16 = np.float16


def _build_nc():
    nc = bacc.Bacc("TRN2", target_bir_lowering=False, debug=False,
                   enable_asserts=False)

    wp = nc.dram_tensor("wp", [N_U, 128, WCOL], _F16, kind="ExternalInput")
    ap = nc.dram_tensor("ap", [2, 128, KT * BS], _F16, kind="ExternalInput")
    ct = nc.dram_tensor("ct", [RT, 128, BS], _F32, kind="ExternalInput")
    h_out = nc.dram_tensor("h_out", [RT, 128, BS], _F32,
                           kind="ExternalOutput")
    c_out = nc.dram_tensor("c_out", [RT, 128, BS], _F32,
                           kind="ExternalOutput")

    with tile.TileContext(nc) as tc:
        with (
            tc.tile_pool(name="const", bufs=1) as cpool,
            tc.tile_pool(name="w", bufs=PREFETCH + 4) as wpool,
            tc.tile_pool(name="gates", bufs=12) as gpool,
            tc.tile_pool(name="cin", bufs=3) as cin_pool,
            tc.tile_pool(name="ew", bufs=4) as epool,
            tc.tile_pool(name="out", bufs=4) as opool,
            tc.tile_pool(name="psum", bufs=8, space="PSUM") as pspool,
        ):
            wt_tiles = {}

            def wt_eng(seq):
                # weight stream alternates sync/scalar queues so the cold
                # startup window delivers two units in parallel
                return nc.sync if seq % 2 == 0 else nc.scalar

            def issue_wt(seq, halves=False):
                t = wpool.tile([128, WCOL], _F16, tag="w")
                eng = wt_eng(seq)
                if halves:
                    hc = (KT // 2) * 128
                    eng.dma_start(out=t[:, 0:hc], in_=wp[seq][:, 0:hc])
                    eng.dma_start(out=t[:, hc:WCOL], in_=wp[seq][:, hc:WCOL])
                else:
                    eng.dma_start(out=t[:], in_=wp[seq])
                wt_tiles[seq] = t

            # PE pre-warm: dummy matmuls on a zeroed tile burn the DVFS
            # p-state ramp while the first operands are in flight.
            warm = cpool.tile([128, 512], _F16, tag="warm")
            nc.gpsimd.memset(warm[:], 0.0)
            wpsum = pspool.tile([128, 512], _F32, tag="pt")
            for _ in range(N_WARM):
                nc.tensor.matmul(wpsum[:], warm[:, 0:128], warm[:],
                                 start=True, stop=True)

            # startup: first units in halves (matmul k=0 waits on 0.2MB),
            # activations as 6KB-row half-tiles on the gpsimd/scalar queues.
            issue_wt(0, halves=True)
            issue_wt(1, halves=True)
            a_sb = {}
            for br in range(2):
                a_tile = cpool.tile([128, KT * BS], _F16, tag=f"a{br}")
                a_sb[br] = a_tile
            hb = (KT // 2) * BS
            nc.gpsimd.dma_start(out=a_sb[0][:, 0:hb], in_=ap[0][:, 0:hb])
            nc.scalar.dma_start(out=a_sb[0][:, hb:KT * BS],
                                in_=ap[0][:, hb:KT * BS])
            for u in range(2, PREFETCH):
                issue_wt(u)
            nc.gpsimd.dma_start(out=a_sb[1][:, 0:hb], in_=ap[1][:, 0:hb])
            nc.scalar.dma_start(out=a_sb[1][:, hb:KT * BS],
                                in_=ap[1][:, hb:KT * BS])

            for r in range(RT):
                ct_t = cin_pool.tile([128, BS], _F32, tag="ct")
                nc.gpsimd.dma_start(out=ct_t[:], in_=ct[r])

                gates = {}
                for br in range(2):
                    for gi, g in enumerate(GATE_ORDER):
                        seq = (r * 2 + br) * 4 + gi
                        if seq + PREFETCH < N_U:
                            issue_wt(seq + PREFETCH)
                        wt = wt_tiles.pop(seq)
                        func = AF.Tanh if g == 2 else AF.Sigmoid
                        a_t = a_sb[br]
                        pt = pspool.tile([128, BS], _F32, tag="pt")
                        for k in range(KT):
                            nc.tensor.matmul(
                                pt[:],
                                wt[:, bass.ts(k, 128)],
                                a_t[:, bass.ts(k, BS)],
                                start=(k == 0),
                                stop=(k == KT - 1),
                            )
                        gt = gpool.tile([128, BS], _F32, tag="gate")
                        bcol = KT * 128
                        nc.scalar.activation(gt[:], pt[:], func,
                                             bias=wt[:, bcol:bcol + 1],
                                             scale=1.0)
                        gates[(br, g)] = gt

                f1, i1, ch1, o1 = (gates[(0, g)] for g in range(4))
                f2, i2, ch2, o2 = (gates[(1, g)] for g in range(4))

                t_a = epool.tile([128, BS], _F32, tag="ta")
                t_b = epool.tile([128, BS], _F32, tag="tb")
                t_c = epool.tile([128, BS], _F32, tag="tc")
                c_new = opool.tile([128, BS], _F32, tag="cn")
                nc.vector.tensor_mul(t_b[:], i1[:], ch1[:])
                nc.vector.tensor_mul(t_c[:], i2[:], ch2[:])
                nc.vector.tensor_add(t_b[:], t_b[:], t_c[:])
                nc.vector.tensor_add(t_a[:], f1[:], f2[:])
                nc.vector.tensor_mul(t_a[:], t_a[:], ct_t[:])
                nc.vector.tensor_add(c_new[:], t_a[:], t_b[:])

                th = epool.tile([128, BS], _F32, tag="th")
                nc.scalar.activation(th[:], c_new[:], AF.Tanh)
                h_new = opool.tile([128, BS], _F32, tag="hn")
                nc.vector.tensor_add(t_a[:], o1[:], o2[:])
                nc.vector.tensor_mul(h_new[:], t_a[:], th[:])

                if r == RT - 1:
                    # fan the tail writes over all three queues
                    q = BS // 2
                    nc.gpsimd.dma_start(out=c_out[r][:, 0:q],
                                        in_=c_new[:, 0:q])
                    nc.sync.dma_start(out=c_out[r][:, q:BS],
                                      in_=c_new[:, q:BS])
                    nc.scalar.dma_start(out=h_out[r][:, 0:q],
                                        in_=h_new[:, 0:q])
                    nc.sync.dma_start(out=h_out[r][:, q:BS],
                                      in_=h_new[:, q:BS])
                else:
                    nc.gpsimd.dma_start(out=c_out[r], in_=c_new[:])
                    nc.gpsimd.dma_start(out=h_out[r], in_=h_new[:])

    nc.compile()
    return nc


_NC_CACHE = None


def _get_nc():
    global _NC_CACHE
    if _NC_CACHE is None:
        _NC_CACHE = _build_nc()
    return _NC_CACHE


def _pack_weights(inputs):
    """-> wp [N_U, 128, KT*128+8] f16, shared by all cores. Column KT*128
    of each unit carries the gate bias for that unit's 128 output rows.

    Unit seq order matches the device loop: (r, br, g in GATE_ORDER).
    Unit layout per partition row kk: [k(KT), m(128)] then bias, where
    wp[u][kk, k*128 + m] = W[u_row m, k*128 + kk].
    """
    units = np.zeros((RT, 2, 4, 128, WCOL), dtype=F16)
    for bi, suffix in enumerate(("_light", "_light_temp")):
        for gi, g in enumerate(GATE_ORDER):
            gname = "fico"[g]
            Wg = inputs[f"w_{gname}{suffix}"].astype(F16)        # [H, K]
            bg = inputs[f"b_{gname}{suffix}"].astype(F16)
            t = Wg.reshape(RT, 128, KT, 128)                     # [r, m, k, kk]
            units[:, bi, gi, :, :KT * 128] = t.transpose(0, 3, 2, 1).reshape(
                RT, 128, KT * 128)
            units[:, bi, gi, :, KT * 128] = bg.reshape(RT, 128)
    return np.ascontiguousarray(units).reshape(N_U, 128, WCOL)


def _pack_core_inputs(inputs, wp, core):
    sl = slice(core * BS, (core + 1) * BS)
    y = inputs["y"][sl]
    out = {"wp": wp}
    a = np.empty((2, 128, KT * BS), dtype=F16)
    for bi, hname in ((0, "h_light"), (1, "h_temp")):
        hx = np.concatenate([inputs[hname][sl], y], axis=1).astype(F16)
        # [p, k, j]: element = hx[j, k*128 + p]
        a2 = hx.reshape(BS, KT, 128).transpose(2, 1, 0)
        a[bi] = np.ascontiguousarray(a2).reshape(128, KT * BS)
    out["ap"] = a
    cl = np.ascontiguousarray(inputs["c_light"][sl].astype(np.float32).T)
    out["ct"] = cl.reshape(RT, 128, BS)
    return out


def make_in_maps(**inputs):
    wp = _pack_weights(inputs)
    return [_pack_core_inputs(inputs, wp, c) for c in range(N_CORES)]


def _unpack_core(res):
    h = res["h_out"].reshape(H, BS).T
    c = res["c_out"].reshape(H, BS).T
    return h, c


def unpack_core0(res0):
    return _unpack_core(res0)


def unpack_results(results):
    h_parts, c_parts = [], []
    for res in results:
        h, c = _unpack_core(res)
        h_parts.append(h)
        c_parts.append(c)
    h_new = np.ascontiguousarray(np.concatenate(h_parts, axis=0),
                                 dtype=np.float32)
    c_new = np.ascontiguousarray(np.concatenate(c_parts, axis=0),
                                 dtype=np.float32)
    return h_new, c_new


def kernel(**inputs):
    inputs = {k: np.asarray(v) for k, v in inputs.items()}
    nc = _get_nc()
    in_maps = make_in_maps(**inputs)
    res = run_bass_kernel_spmd(nc, in_maps, list(range(N_CORES)))
    return unpack_results(res.results)


# revision 13
# speedup vs baseline: 1.0757x; 1.0031x over previous
"""Trainium2 Bass kernel for the dual-branch CustomLSTMCell.

Math (reference):
    hx_l = [h_light | y]  [B, H+I]     hx_t = [h_temp | y]
    z_br = hx_br @ W_br.T + b_br       (W_br = vstack(w_f,w_i,w_c,w_o) [4H, H+I])
    f,i,ch,o = sigmoid/sigmoid/tanh/sigmoid splits of z_br
    c_new = (f1 + f2) * c_light + i1*ch1 + i2*ch2      (c_temp is unused)
    h_new = (o1 + o2) * tanh(c_new)

Strategy: pure data-parallel over 8 NeuronCores — batch 4096 -> 8 x 512,
weights replicated. Per core we compute z.T tiles: psum[zcol 128, batch 512]
= Wtile[K=128, M=128].T @ hxT[K=128, N=512], accumulated over K=1536 (12
k-tiles) in fp16 (1 PE cycle/row; fp32 would be 4x slower; fp8-DoubleRow's
2x FLOP rate cannot pay for the 3x FLOPs a split-precision scheme needs to
stay under the accuracy gate — measured 271us vs this design's 164us PE
floor). Gate bias + sigmoid/tanh run on the Scalar engine straight out of
PSUM (bias rides in the last column of each weight unit, so it needs no
separate descriptor-heavy DMA), the LSTM cell elementwise runs on the
Vector engine in fp32, results DMA out transposed, host un-transposes.

DMA-queue economics drive the schedule: a DMA costs one descriptor per
partition row (~15ns dispatch each), so only wide-row transfers are fast,
and the queues deliver slowly for the first ~15us. The weight stream is
split across the sync and scalar queues (even/odd units, 3KB rows, first
units halved so matmul k=0 waits on 0.2MB), activations move as 6KB-row
half-tiles, c_light and the output writes ride the gpsimd queue, and the
final row-tile's outputs fan out over all three queues to shorten the
tail. A PE pre-warm (dummy matmuls) burns the DVFS p-state ramp while the
first operands are in flight. The per-branch gate order is (i, c, f, o) so
the output gate of the temp branch — the only input of the final h_new
chain — finishes last and the tail after the final matmul is short.
"""

import os
import sys

for _p in ("/opt/trn_rl_repo",):
    if os.path.isdir(_p) and _p not in sys.path:
        sys.path.append(_p)

import numpy as np

import concourse.bass as bass
import concourse.mybir as mybir
import concourse.tile as tile
from concourse import bacc
from concourse.bass_utils import run_bass_kernel_spmd

B, I, H = 4096, 512, 1024
N_CORES = 8
BS = B // N_CORES          # 512 batch rows per core
RT = H // 128              # 8 hidden row-tiles
K = H + I                  # 1536 contraction
KT = K // 128              # 12 k-tiles
N_U = RT * 2 * 4           # 64 weight units: (r, branch, gate)
WCOL = KT * 128 + 8        # weight unit cols: 12 k-tiles + bias col
GATE_ORDER = (1, 2, 0, 3)  # i, c, f, o
PREFETCH = 8               # weight units in flight ahead of use
N_WARM = 11                # PE pre-warm matmuls (512-cycle each)

_F32 = mybir.dt.float32
_F16 = mybir.dt.float16
AF = mybir.ActivationFunctionType
F16 = np.float16


def _build_nc():
    nc = bacc.Bacc("TRN2", target_bir_lowering=False, debug=False,
                   enable_asserts=False)

    wp = nc.dram_tensor("wp", [N_U, 128, WCOL], _F16, kind="ExternalInput")
    ap = nc.dram_tensor("ap", [2, 128, KT * BS], _F16, kind="ExternalInput")
    ct = nc.dram_tensor("ct", [RT, 128, BS], _F32, kind="ExternalInput")
    h_out = nc.dram_tensor("h_out", [RT, 128, BS], _F32,
                           kind="ExternalOutput")
    c_out = nc.dram_tensor("c_out", [RT, 128, BS], _F32,
                           kind="ExternalOutput")

    with tile.TileContext(nc) as tc:
        with (
            tc.tile_pool(name="const", bufs=1) as cpool,
            tc.tile_pool(name="w", bufs=PREFETCH + 4) as wpool,
            tc.tile_pool(name="gates", bufs=12) as gpool,
            tc.tile_pool(name="cin", bufs=3) as cin_pool,
            tc.tile_pool(name="ew", bufs=4) as epool,
            tc.tile_pool(name="out", bufs=4) as opool,
            tc.tile_pool(name="psum", bufs=8, space="PSUM") as pspool,
        ):
            wt_tiles = {}

            def wt_eng(seq):
                # sync queue carries the whole weight stream: its trigger
                # instructions never queue behind activations or memsets
                return nc.sync

            def issue_wt(seq, halves=False):
                t = wpool.tile([128, WCOL], _F16, tag="w")
                eng = wt_eng(seq)
                if halves:
                    hc = (KT // 2) * 128
                    eng.dma_start(out=t[:, 0:hc], in_=wp[seq][:, 0:hc])
                    eng.dma_start(out=t[:, hc:WCOL], in_=wp[seq][:, hc:WCOL])
                else:
                    eng.dma_start(out=t[:], in_=wp[seq])
                wt_tiles[seq] = t

            # PE pre-warm: dummy matmuls on a zeroed tile burn the DVFS
            # p-state ramp while the first operands are in flight.
            warm = cpool.tile([128, 512], _F16, tag="warm")
            nc.gpsimd.memset(warm[:], 0.0)
            wpsum = pspool.tile([128, 512], _F32, tag="pt")
            for _ in range(N_WARM):
                nc.tensor.matmul(wpsum[:], warm[:, 0:128], warm[:],
                                 start=True, stop=True)

            # startup: first units in halves (matmul k=0 waits on 0.2MB),
            # activations as 6KB-row half-tiles on the gpsimd/scalar queues.
            a_sb = {}
            for br in range(2):
                a_tile = cpool.tile([128, KT * BS], _F16, tag=f"a{br}")
                a_sb[br] = a_tile
            hb = (KT // 2) * BS
            nc.gpsimd.dma_start(out=a_sb[0][:, 0:hb], in_=ap[0][:, 0:hb])
            nc.scalar.dma_start(out=a_sb[0][:, hb:KT * BS],
                                in_=ap[0][:, hb:KT * BS])
            issue_wt(0, halves=True)
            issue_wt(1, halves=True)
            nc.gpsimd.dma_start(out=a_sb[1][:, 0:hb], in_=ap[1][:, 0:hb])
            nc.scalar.dma_start(out=a_sb[1][:, hb:KT * BS],
                                in_=ap[1][:, hb:KT * BS])
            for u in range(2, PREFETCH):
                issue_wt(u)

            for r in range(RT):
                ct_t = cin_pool.tile([128, BS], _F32, tag="ct")
                nc.gpsimd.dma_start(out=ct_t[:], in_=ct[r])

                gates = {}
                for br in range(2):
                    for gi, g in enumerate(GATE_ORDER):
                        seq = (r * 2 + br) * 4 + gi
                        if seq + PREFETCH < N_U:
                            issue_wt(seq + PREFETCH)
                        wt = wt_tiles.pop(seq)
                        func = AF.Tanh if g == 2 else AF.Sigmoid
                        a_t = a_sb[br]
                        pt = pspool.tile([128, BS], _F32, tag="pt")
                        for k in range(KT):
                            nc.tensor.matmul(
                                pt[:],
                                wt[:, bass.ts(k, 128)],
                                a_t[:, bass.ts(k, BS)],
                                start=(k == 0),
                                stop=(k == KT - 1),
                            )
                        gt = gpool.tile([128, BS], _F32, tag="gate")
                        bcol = KT * 128
                        nc.scalar.activation(gt[:], pt[:], func,
                                             bias=wt[:, bcol:bcol + 1],
                                             scale=1.0)
                        gates[(br, g)] = gt

                f1, i1, ch1, o1 = (gates[(0, g)] for g in range(4))
                f2, i2, ch2, o2 = (gates[(1, g)] for g in range(4))

                t_a = epool.tile([128, BS], _F32, tag="ta")
                t_b = epool.tile([128, BS], _F32, tag="tb")
                t_c = epool.tile([128, BS], _F32, tag="tc")
                c_new = opool.tile([128, BS], _F32, tag="cn")
                nc.vector.tensor_mul(t_b[:], i1[:], ch1[:])
                nc.vector.tensor_mul(t_c[:], i2[:], ch2[:])
                nc.vector.tensor_add(t_b[:], t_b[:], t_c[:])
                nc.vector.tensor_add(t_a[:], f1[:], f2[:])
                nc.vector.tensor_mul(t_a[:], t_a[:], ct_t[:])
                nc.vector.tensor_add(c_new[:], t_a[:], t_b[:])

                th = epool.tile([128, BS], _F32, tag="th")
                nc.scalar.activation(th[:], c_new[:], AF.Tanh)
                h_new = opool.tile([128, BS], _F32, tag="hn")
                nc.vector.tensor_add(t_a[:], o1[:], o2[:])
                nc.vector.tensor_mul(h_new[:], t_a[:], th[:])

                if r == RT - 1:
                    # fan the tail writes over all three queues
                    q = BS // 2
                    qq = BS // 4
                    nc.gpsimd.dma_start(out=c_out[r][:, 0:q],
                                        in_=c_new[:, 0:q])
                    nc.sync.dma_start(out=c_out[r][:, q:BS],
                                      in_=c_new[:, q:BS])
                    nc.scalar.dma_start(out=h_out[r][:, 0:qq],
                                        in_=h_new[:, 0:qq])
                    nc.sync.dma_start(out=h_out[r][:, qq:2 * qq],
                                      in_=h_new[:, qq:2 * qq])
                    nc.gpsimd.dma_start(out=h_out[r][:, 2 * qq:3 * qq],
                                        in_=h_new[:, 2 * qq:3 * qq])
                    nc.sync.dma_start(out=h_out[r][:, 3 * qq:BS],
                                      in_=h_new[:, 3 * qq:BS])
                else:
                    nc.gpsimd.dma_start(out=c_out[r], in_=c_new[:])
                    nc.gpsimd.dma_start(out=h_out[r], in_=h_new[:])

    nc.compile()
    return nc


_NC_CACHE = None


def _get_nc():
    global _NC_CACHE
    if _NC_CACHE is None:
        _NC_CACHE = _build_nc()
    return _NC_CACHE


def _pack_weights(inputs):
    """-> wp [N_U, 128, KT*128+8] f16, shared by all cores. Column KT*128
    of each unit carries the gate bias for that unit's 128 output rows.

    Unit seq order matches the device loop: (r, br, g in GATE_ORDER).
    Unit layout per partition row kk: [k(KT), m(128)] then bias, where
    wp[u][kk, k*128 + m] = W[u_row m, k*128 + kk].
    """
    units = np.zeros((RT, 2, 4, 128, WCOL), dtype=F16)
    for bi, suffix in enumerate(("_light", "_light_temp")):
        for gi, g in enumerate(GATE_ORDER):
            gname = "fico"[g]
            Wg = inputs[f"w_{gname}{suffix}"].astype(F16)        # [H, K]
            bg = inputs[f"b_{gname}{suffix}"].astype(F16)
            t = Wg.reshape(RT, 128, KT, 128)                     # [r, m, k, kk]
            units[:, bi, gi, :, :KT * 128] = t.transpose(0, 3, 2, 1).reshape(
                RT, 128, KT * 128)
            units[:, bi, gi, :, KT * 128] = bg.reshape(RT, 128)
    return np.ascontiguousarray(units).reshape(N_U, 128, WCOL)


def _pack_core_inputs(inputs, wp, core):
    sl = slice(core * BS, (core + 1) * BS)
    y = inputs["y"][sl]
    out = {"wp": wp}
    a = np.empty((2, 128, KT * BS), dtype=F16)
    for bi, hname in ((0, "h_light"), (1, "h_temp")):
        hx = np.concatenate([inputs[hname][sl], y], axis=1).astype(F16)
        # [p, k, j]: element = hx[j, k*128 + p]
        a2 = hx.reshape(BS, KT, 128).transpose(2, 1, 0)
        a[bi] = np.ascontiguousarray(a2).reshape(128, KT * BS)
    out["ap"] = a
    cl = np.ascontiguousarray(inputs["c_light"][sl].astype(np.float32).T)
    out["ct"] = cl.reshape(RT, 128, BS)
    return out


def make_in_maps(**inputs):
    wp = _pack_weights(inputs)
    return [_pack_core_inputs(inputs, wp, c) for c in range(N_CORES)]


def _unpack_core(res):
    h = res["h_out"].reshape(H, BS).T
    c = res["c_out"].reshape(H, BS).T
    return h, c


def unpack_core0(res0):
    return _unpack_core(res0)


def unpack_results(results):
    h_parts, c_parts = [], []
    for res in results:
        h, c = _unpack_core(res)
        h_parts.append(h)
        c_parts.append(c)
    h_new = np.ascontiguousarray(np.concatenate(h_parts, axis=0),
                                 dtype=np.float32)
    c_new = np.ascontiguousarray(np.concatenate(c_parts, axis=0),
                                 dtype=np.float32)
    return h_new, c_new


def kernel(**inputs):
    inputs = {k: np.asarray(v) for k, v in inputs.items()}
    nc = _get_nc()
    in_maps = make_in_maps(**inputs)
    res = run_bass_kernel_spmd(nc, in_maps, list(range(N_CORES)))
    return unpack_results(res.results)


# revision 14
# speedup vs baseline: 1.1178x; 1.0391x over previous
"""Trainium2 Bass kernel for the dual-branch CustomLSTMCell.

Math (reference):
    hx_l = [h_light | y]  [B, H+I]     hx_t = [h_temp | y]
    z_br = hx_br @ W_br.T + b_br       (W_br = vstack(w_f,w_i,w_c,w_o) [4H, H+I])
    f,i,ch,o = sigmoid/sigmoid/tanh/sigmoid splits of z_br
    c_new = (f1 + f2) * c_light + i1*ch1 + i2*ch2      (c_temp is unused)
    h_new = (o1 + o2) * tanh(c_new)

Strategy: pure data-parallel over 8 NeuronCores -- batch 4096 -> 8 x 512,
weights replicated. Per core we compute z.T tiles: psum[zcol 128, batch 512]
= Wtile[K=128, M=128].T @ hxT[K=128, N=512], accumulated over K=1536 (12
k-tiles), in fp16 (1 PE cycle/row like bf16 -- fp32 would be 4x slower -- but
with 8x finer mantissa). Gate bias + sigmoid/tanh run on the Scalar engine
straight out of PSUM (bias is per-partition in this transposed layout), the
LSTM cell elementwise runs on the Vector engine in fp32, results DMA out
transposed, and the host un-transposes. All transposes/casts happen host-side
so every device DMA is a contiguous 2D partition-major stream.

Schedule notes: weight DMAs are issued with a software prefetch distance of
PREFETCH tiles so the first matmul's operands land within ~2us of kernel
start; the per-branch gate order is (i, c, f, o) so the output gate of the
temp branch - the only input of the final h_new chain - finishes last and
the tail after the final matmul is short. The final row-tile's h/c writes
fan out over all three DMA queues (sync/scalar/gpsimd) so the tail drain
is parallel rather than serial on the sync queue.
"""

import os
import sys

for _p in ("/opt/trn_rl_repo",):
    if os.path.isdir(_p) and _p not in sys.path:
        sys.path.append(_p)

import numpy as np

import concourse.bass as bass
import concourse.mybir as mybir
import concourse.tile as tile
from concourse import bacc
from concourse.bass_utils import run_bass_kernel_spmd

B, I, H = 4096, 512, 1024
N_CORES = 8
BS = B // N_CORES          # 512 batch rows per core
K = H + I                  # 1536 contraction
KT = K // 128              # 12 k-tiles
RT = H // 128              # 8 zcol (hidden) tiles per gate
N_W = RT * 2 * 4           # 64 weight tiles: (r, branch, gate)
GATE_ORDER = (1, 2, 0, 3)  # i, c, f, o
PREFETCH = 8               # weight tiles in flight ahead of use

_F32 = mybir.dt.float32
_F16 = mybir.dt.float16
AF = mybir.ActivationFunctionType
F16 = np.float16


def _build_nc():
    nc = bacc.Bacc("TRN2", target_bir_lowering=False, debug=False,
                   enable_asserts=False)

    wp = nc.dram_tensor("wp", [N_W, 128, KT * 128], _F16, kind="ExternalInput")
    a_l = nc.dram_tensor("a_l", [128, KT * BS], _F16, kind="ExternalInput")
    a_t = nc.dram_tensor("a_t", [128, KT * BS], _F16, kind="ExternalInput")
    bp = nc.dram_tensor("bp", [128, N_W], _F32, kind="ExternalInput")
    ct = nc.dram_tensor("ct", [RT, 128, BS], _F32, kind="ExternalInput")
    h_out = nc.dram_tensor("h_out", [RT, 128, BS], _F32, kind="ExternalOutput")
    c_out = nc.dram_tensor("c_out", [RT, 128, BS], _F32, kind="ExternalOutput")

    with tile.TileContext(nc) as tc:
        with (
            tc.tile_pool(name="const", bufs=1) as cpool,
            tc.tile_pool(name="w", bufs=PREFETCH + 4) as wpool,
            tc.tile_pool(name="gates", bufs=16) as gpool,
            tc.tile_pool(name="cin", bufs=2) as cin_pool,
            tc.tile_pool(name="ew", bufs=4) as epool,
            tc.tile_pool(name="out", bufs=4) as opool,
            tc.tile_pool(name="psum", bufs=8, space="PSUM") as pspool,
        ):
            wt_tiles = {}

            def issue_wt(seq, eng=None):
                t = wpool.tile([128, KT * 128], _F16, tag="w")
                (eng or nc.sync).dma_start(out=t[:], in_=wp[seq])
                wt_tiles[seq] = t

            # PE pre-warm: dummy matmuls on a zeroed tile start the HAM
            # clock-gate busy window while the first operands are in flight.
            warm = cpool.tile([128, BS], _F16, tag="warm")
            nc.gpsimd.memset(warm[:], 0.0)
            wpsum = pspool.tile([128, BS], _F32, tag="pt")
            for _ in range(8):
                nc.tensor.matmul(wpsum[:], warm[:, 0:128], warm[:],
                                 start=True, stop=True)

            # startup: first matmul's operands move immediately, triggers
            # spread across sequencers so they fire in parallel.
            issue_wt(0, nc.scalar)
            a_sb = []
            for name, src in (("al", a_l), ("at", a_t)):
                t = cpool.tile([128, KT * BS], _F16, tag=name)
                a_sb.append(t)
            nc.gpsimd.dma_start(out=a_sb[0][:, bass.ts(0, BS)],
                                in_=a_l[:, bass.ts(0, BS)])
            issue_wt(1)
            for k in range(1, KT):
                nc.sync.dma_start(out=a_sb[0][:, bass.ts(k, BS)],
                                  in_=a_l[:, bass.ts(k, BS)])
                if k < 3:
                    issue_wt(k + 1)
            for k in range(KT):
                nc.sync.dma_start(out=a_sb[1][:, bass.ts(k, BS)],
                                  in_=a_t[:, bass.ts(k, BS)])
                if k < 4:
                    issue_wt(k + 4)
            bias_sb = cpool.tile([128, N_W], _F32, tag="bias")
            nc.sync.dma_start(out=bias_sb[:], in_=bp[:])

            seq = 0  # sequential weight-tile index (matches host pack order)
            for r in range(RT):
                ct_t = cin_pool.tile([128, BS], _F32, tag="ct")
                nc.sync.dma_start(out=ct_t[:], in_=ct[r])

                gates = {}
                for br in range(2):
                    for g in GATE_ORDER:
                        if seq + PREFETCH < N_W:
                            issue_wt(seq + PREFETCH)
                        idx = (r * 2 + br) * 4 + g
                        wt = wt_tiles.pop(seq)
                        pt = pspool.tile([128, BS], _F32, tag="pt")
                        for k in range(KT):
                            nc.tensor.matmul(
                                pt[:],
                                wt[:, bass.ts(k, 128)],
                                a_sb[br][:, bass.ts(k, BS)],
                                start=(k == 0),
                                stop=(k == KT - 1),
                            )
                        gt = gpool.tile([128, BS], _F32, tag="gate")
                        func = AF.Tanh if g == 2 else AF.Sigmoid
                        nc.scalar.activation(gt[:], pt[:], func,
                                             bias=bias_sb[:, idx:idx + 1],
                                             scale=1.0)
                        gates[(br, g)] = gt
                        seq += 1

                f1, i1, ch1, o1 = (gates[(0, g)] for g in range(4))
                f2, i2, ch2, o2 = (gates[(1, g)] for g in range(4))

                t_a = epool.tile([128, BS], _F32, tag="ta")
                t_b = epool.tile([128, BS], _F32, tag="tb")
                t_c = epool.tile([128, BS], _F32, tag="tc")
                c_new = opool.tile([128, BS], _F32, tag="cn")
                nc.vector.tensor_mul(t_b[:], i1[:], ch1[:])
                nc.vector.tensor_mul(t_c[:], i2[:], ch2[:])
                nc.vector.tensor_add(t_b[:], t_b[:], t_c[:])
                nc.vector.tensor_add(t_a[:], f1[:], f2[:])        # f1+f2
                nc.vector.tensor_mul(t_a[:], t_a[:], ct_t[:])     # *c_light
                nc.vector.tensor_add(c_new[:], t_a[:], t_b[:])

                th = epool.tile([128, BS], _F32, tag="th")
                nc.scalar.activation(th[:], c_new[:], AF.Tanh)
                h_new = opool.tile([128, BS], _F32, tag="hn")
                nc.vector.tensor_add(t_a[:], o1[:], o2[:])        # o1+o2
                nc.vector.tensor_mul(h_new[:], t_a[:], th[:])

                if r == RT - 1:
                    # parallel tail drain: quarters of h and halves of c
                    # spread over the sync/scalar/gpsimd queues
                    q2 = BS // 2
                    q4 = BS // 4
                    nc.gpsimd.dma_start(out=c_out[r][:, 0:q2],
                                        in_=c_new[:, 0:q2])
                    nc.sync.dma_start(out=c_out[r][:, q2:BS],
                                      in_=c_new[:, q2:BS])
                    nc.scalar.dma_start(out=h_out[r][:, 0:q4],
                                        in_=h_new[:, 0:q4])
                    nc.sync.dma_start(out=h_out[r][:, q4:2 * q4],
                                      in_=h_new[:, q4:2 * q4])
                    nc.gpsimd.dma_start(out=h_out[r][:, 2 * q4:3 * q4],
                                        in_=h_new[:, 2 * q4:3 * q4])
                    nc.sync.dma_start(out=h_out[r][:, 3 * q4:BS],
                                      in_=h_new[:, 3 * q4:BS])
                else:
                    nc.sync.dma_start(out=c_out[r], in_=c_new[:])
                    nc.sync.dma_start(out=h_out[r], in_=h_new[:])

    nc.compile()
    return nc


_NC_CACHE = None


def _get_nc():
    global _NC_CACHE
    if _NC_CACHE is None:
        _NC_CACHE = _build_nc()
    return _NC_CACHE


def _pack_weights(inputs):
    """-> wp [N_W, 128, KT*128] f16, bp [128, N_W] f32 (shared by all cores).

    Weight-tile seq order must match the device loop: (r, br, g in GATE_ORDER).
    """
    wps, bps = [], []
    for suffix in ("_light", "_light_temp"):
        Wc = np.concatenate([inputs["w_f" + suffix], inputs["w_i" + suffix],
                             inputs["w_c" + suffix], inputs["w_o" + suffix]],
                            axis=0)                       # [4H, K]
        bc = np.concatenate([inputs["b_f" + suffix], inputs["b_i" + suffix],
                             inputs["b_c" + suffix], inputs["b_o" + suffix]])
        # tile (r, g): sbuf[kk, k*128 + m] = lhsT_k[kk, m]
        #            = Wc[g*1024 + r*128 + m, k*128 + kk]
        Wt = Wc.reshape(4, RT, 128, KT, 128)              # [g, r, m, k, kk]
        Wt = Wt.transpose(1, 0, 4, 3, 2)                  # [r, g, kk, k, m]
        wps.append(np.ascontiguousarray(Wt).astype(F16))
        bps.append(bc.reshape(4, RT, 128).transpose(2, 1, 0))  # [p, r, g]
    wp = np.stack(wps, axis=1)                            # [r, br, g, kk, k, m]
    wp = wp[:, :, GATE_ORDER]                             # device consumption order
    wp = np.ascontiguousarray(wp).reshape(N_W, 128, KT * 128)
    bp = np.stack(bps, axis=2)                            # [p, r, br, g]
    bp = np.ascontiguousarray(bp).reshape(128, N_W).astype(np.float32)
    return wp, bp


def _pack_core_inputs(inputs, wp, bp, core):
    sl = slice(core * BS, (core + 1) * BS)
    y = inputs["y"][sl]
    out = {"wp": wp, "bp": bp}
    for name, h in (("a_l", inputs["h_light"][sl]), ("a_t", inputs["h_temp"][sl])):
        hx = np.concatenate([h, y], axis=1).astype(F16)   # [BS, K]
        # sbuf[p, k*BS + j] = hx[j, k*128 + p]
        a2 = hx.reshape(BS, KT, 128).transpose(2, 1, 0)
        out[name] = np.ascontiguousarray(a2).reshape(128, KT * BS)
    cl = np.ascontiguousarray(inputs["c_light"][sl].astype(np.float32).T)
    out["ct"] = cl.reshape(RT, 128, BS)
    return out


def make_in_maps(**inputs):
    wp, bp = _pack_weights(inputs)
    return [_pack_core_inputs(inputs, wp, bp, c) for c in range(N_CORES)]


def unpack_core0(res0):
    h0 = res0["h_out"].reshape(H, BS).T
    c0 = res0["c_out"].reshape(H, BS).T
    return h0, c0


def unpack_results(results):
    h_parts, c_parts = [], []
    for res in results:
        h_parts.append(res["h_out"].reshape(H, BS).T)
        c_parts.append(res["c_out"].reshape(H, BS).T)
    h_new = np.ascontiguousarray(np.concatenate(h_parts, axis=0), dtype=np.float32)
    c_new = np.ascontiguousarray(np.concatenate(c_parts, axis=0), dtype=np.float32)
    return h_new, c_new


def kernel(**inputs):
    inputs = {k: np.asarray(v) for k, v in inputs.items()}
    nc = _get_nc()
    in_maps = make_in_maps(**inputs)
    res = run_bass_kernel_spmd(nc, in_maps, list(range(N_CORES)))
    return unpack_results(res.results)
